# revision 29
# baseline (speedup 1.0000x reference)
"""Distributed Trainium2 kernel for nn_Attention_79207786873592.

Full attention block: qkv projection -> RMSNorm(q,k) -> RoPE -> SDPA -> wo.
B=4, L=2048, D=2048, H=16 heads, head_dim=128, fp32 I/O (bf16 compute).

Sharding: 8-way tensor-parallel over heads. Each core owns 2 heads:
  - computes its 768-row slice of the qkv projection,
  - a tiny AllReduce (64KB) produces the global per-token sum-of-squares for
    the full-D RMSNorm of q and k,
  - local RoPE + SDPA on its 2 heads,
  - AllGather of the per-head attention context (per batch half, pipelined),
  - local 256-column slice of the wo projection.
Host side pre-transposes x and the weights into [contract_dim, out] layouts so
no on-device transposes are needed, and folds the norm weights into per-head
RoPE tables. Within each head, q/k output dims are permuted (evens, then odds)
so RoPE pair mixing becomes partition-block operations on the vector engine.

Perf structure:
  - qkv processes chunks in pairs sharing each stationary weight load.
  - scores for two 512-col i-chunks land in one [128,1024] PSUM pair tile so
    a single EXP instruction covers both; softmax partials accumulate on the
    vector engine (even tiles) and gpsimd (odd tiles).
  - softmax epilogues (denominator reduce / reciprocal / normalize) are
    deferred and emitted inside the next block's matmul stream so the
    in-order PE queue never stalls on vector/scalar results.
  - broadcasts use fp32r / bf16 matmuls (1 cycle/row instead of 4).
  - wo is emitted in slices interleaved with the next batch's attention and
    consumes per-half AllGathers to shorten the tail.
"""

import sys

sys.path.insert(0, "/opt/trn_rl_repo")

import numpy as np
import ml_dtypes

import concourse.bass as bass
import concourse.tile as tile
import concourse.mybir as mybir
from concourse import bacc

B, L, D, H = 4, 2048, 2048, 16
HD = D // H              # 128
NC = 8                   # cores
HPC = H // NC            # 2 heads per core
DQ = HPC * HD            # 256 rows of q/k/v per core
T = B * L                # 8192 tokens
EPS = 1e-5
CH = 512                 # phase-1 token chunk
NCH = T // CH            # 16
NDT = D // 128           # 16 contraction tiles
NJ = L // 128            # 16 k-tiles per batch
BF = mybir.dt.bfloat16
F32 = mybir.dt.float32
F32R = mybir.dt.float32r
BF_NP = ml_dtypes.bfloat16

_CACHE = {}


def build_nc():
    nc = bacc.Bacc("TRN2", target_bir_lowering=False, debug=False, num_devices=NC)

    xT = nc.dram_tensor("xT", [D, T], BF, kind="ExternalInput").ap()
    wqkvT = nc.dram_tensor("wqkvT", [D, 3 * DQ], BF, kind="ExternalInput").ap()
    woT = nc.dram_tensor("woT", [D, DQ], BF, kind="ExternalInput").ap()
    TAB_NAMES = [p + sfx for p in ("tq", "tk")
                 for sfx in ("ce", "so", "co", "se")]
    tab_ext = {nm: nc.dram_tensor(nm, [HD, L], BF, kind="ExternalInput").ap()
               for nm in TAB_NAMES}
    outT = nc.dram_tensor("outT", [DQ, T], F32, kind="ExternalOutput").ap()

    with tile.TileContext(nc) as tc:
        with tc.tile_pool(name="dram", bufs=1, space="DRAM") as dram, \
             tc.tile_pool(name="consts", bufs=1) as consts:

            q_dram = dram.tile([DQ, T], BF, tag="q_dram")
            k_dram = dram.tile([DQ, T], BF, tag="k_dram")
            # v in [partition, head, token-tile, head-dim] layout so both the
            # phase-1 writes and the per-head attention reads are contiguous
            v_dram = dram.tile([128, HPC, T // 128, 128], BF, tag="v_dram")
            ss_in = [dram.tile([2, L], F32, tag=f"ss_in{b}", name=f"ss_in{b}")
                     for b in range(B)]
            ss_out = [dram.tile([2, L], F32, tag=f"ss_out{b}", name=f"ss_out{b}",
                                addr_space="Shared") for b in range(B)]
            ss_fin = [dram.tile([2, L], F32, tag=f"ss_fin{b}", name=f"ss_fin{b}")
                      for b in range(B)]
            # per-half context buffers: half h covers token cols h*1024..
            ctx_b = [[dram.tile([DQ, L // 2], BF, tag=f"ctxb{b}_{h}",
                                name=f"ctxb{b}_{h}") for h in range(2)]
                     for b in range(B)]
            ctx_g = [[dram.tile([NC * DQ, L // 2], BF, tag=f"ctxg{b}_{h}",
                                name=f"ctxg{b}_{h}", addr_space="Shared")
                      for h in range(2)] for b in range(B)]

            ones_c = consts.tile([128, 1], F32, tag="ones_c")
            nc.vector.memset(ones_c[:], 1.0)
            ones_cb = consts.tile([128, 1], BF, tag="ones_cb")
            nc.vector.memset(ones_cb[:], 1.0)
            ones_r = consts.tile([1, 128], F32, tag="ones_r")
            nc.vector.memset(ones_r[:], 1.0)
            ones_rb = consts.tile([1, 128], BF, tag="ones_rb")
            nc.vector.memset(ones_rb[:], 1.0)
            sc2 = consts.tile([128, 1], F32, tag="sc2")
            nc.vector.memset(sc2[0:64, :], 1.0 / HD)
            nc.vector.memset(sc2[64:128, :], 1.0)
            eps2 = consts.tile([128, 1], F32, tag="eps2")
            nc.vector.memset(eps2[:], EPS)
            ident = consts.tile([128, 128], BF, tag="ident")
            from concourse.masks import make_identity
            make_identity(nc, ident[:])

            # ---------------- Phase 1: qkv + sumsq + rope ----------------
            with tc.tile_pool(name="wres1", bufs=1) as wres1, \
                 tc.tile_pool(name="xc", bufs=4) as xcp, \
                 tc.tile_pool(name="p1sb", bufs=2) as p1sb, \
                 tc.tile_pool(name="p1ss", bufs=3) as p1ss, \
                 tc.tile_pool(name="qkps", bufs=6, space="PSUM") as qkps, \
                 tc.tile_pool(name="vps", bufs=1, space="PSUM") as vps, \
                 tc.tile_pool(name="ssps", bufs=1, space="PSUM") as ssps:

                # resident qkv weights; issue after nothing, before tables,
                # and prefetch the first two x chunks before the tables so
                # the first matmul starts as early as possible.
                # weight/x/table loads split so early consumers only wait on
                # the slice they read, not the whole tensor
                wq_sb = wres1.tile([128, NDT * 3 * DQ], BF, tag="wq_sb")
                wqr = wq_sb.rearrange("p (a w) -> p a w", a=NDT)
                wqs = wqkvT.rearrange("(a p) w -> p a w", p=128)
                for dg in range(4):
                    nc.sync.dma_start(wqr[:, 4 * dg:4 * (dg + 1), :],
                                      wqs[:, 4 * dg:4 * (dg + 1), :])

                xTr = xT.rearrange("(a p) t -> p a t", p=128)

                def load_xc(c):
                    xc = xcp.tile([128, NDT * CH], BF, tag="xc", name=f"xc{c % 4}")
                    xcr = xc.rearrange("p (a t) -> p a t", a=NDT)
                    for dg in range(2):
                        nc.sync.dma_start(
                            xcr[:, 8 * dg:8 * (dg + 1), :],
                            xTr[:, 8 * dg:8 * (dg + 1), c * CH:(c + 1) * CH])
                    return xc

                xc_pref = [load_xc(0), load_xc(1)]

                tabs = {}
                for nm, t in tab_ext.items():
                    tt = wres1.tile([128, L], BF, tag=nm + "_sb", name=nm + "_sb")
                    for cg_ in range(4):
                        nc.sync.dma_start(tt[:, cg_ * CH:(cg_ + 1) * CH],
                                          t[:, cg_ * CH:(cg_ + 1) * CH])
                    tabs[nm] = tt

                w3 = 3 * DQ

                def qk_mms(xcA, xcB, side):
                    """Emit q-or-k matmuls for a chunk pair; returns
                    [psE_A, psE_B, psO_A, psO_B]."""
                    out = []
                    for eo in range(2):
                        m = side * 2 + eo
                        psA = qkps.tile([128, CH], F32, tag="qk", name="psA")
                        psB = qkps.tile([128, CH], F32, tag="qk", name="psB")
                        for d in range(NDT):
                            wsl = wq_sb[:, d * w3 + m * 128:d * w3 + (m + 1) * 128]
                            nc.tensor.matmul(psA[:], wsl, xcA[:, d * CH:(d + 1) * CH],
                                             start=(d == 0), stop=(d == NDT - 1))
                            nc.tensor.matmul(psB[:], wsl, xcB[:, d * CH:(d + 1) * CH],
                                             start=(d == 0), stop=(d == NDT - 1))
                        out.append((psA, psB))
                    (psEA, psEB), (psOA, psOB) = out
                    return [(psEA, psOA), (psEB, psOB)]

                def sumsq(ps_pair, side, c):
                    psE, psO = ps_pair
                    ssp = ssps.tile([1, CH], F32, tag="ssp", name="ssp")
                    for eo, ps in enumerate((psE, psO)):
                        sq = p1sb.tile([128, CH], BF, tag="sq", bufs=3)
                        nc.scalar.square(sq[:], ps[:])
                        nc.tensor.matmul(ssp[:], ones_cb[:], sq[:],
                                         start=(eo == 0), stop=(eo == 1))
                    ssr = p1sb.tile([1, CH], F32, tag="ssr", bufs=2)
                    nc.scalar.copy(ssr[:], ssp[:])
                    nc.sync.dma_start(
                        ss_in[c // 4][side:side + 1,
                                      (c % 4) * CH:(c % 4 + 1) * CH],
                        ssr[:])

                def rope(ps_pair, side, c):
                    psE, psO = ps_pair
                    lc = c % 4
                    tsl = slice(lc * CH, (lc + 1) * CH)
                    csl = slice(c * CH, (c + 1) * CH)
                    pre = "tq" if side == 0 else "tk"
                    ta = p1sb.tile([128, CH], F32, tag="ta")
                    nc.vector.tensor_mul(ta[:], psE[:], tabs[pre + "ce"][:, tsl])
                    tb = p1sb.tile([128, CH], F32, tag="tb")
                    nc.vector.tensor_mul(tb[:], psO[:], tabs[pre + "so"][:, tsl])
                    roE = p1sb.tile([128, CH], BF, tag="roE")
                    nc.vector.tensor_sub(roE[:], ta[:], tb[:])
                    tc_ = p1sb.tile([128, CH], F32, tag="tc_")
                    nc.vector.tensor_mul(tc_[:], psO[:], tabs[pre + "co"][:, tsl])
                    td = p1sb.tile([128, CH], F32, tag="td")
                    nc.vector.tensor_mul(td[:], psE[:], tabs[pre + "se"][:, tsl])
                    roO = p1sb.tile([128, CH], BF, tag="roO")
                    nc.vector.tensor_add(roO[:], tc_[:], td[:])
                    dst = q_dram if side == 0 else k_dram
                    for hl in range(HPC):
                        nc.sync.dma_start(
                            dst[hl * 128:hl * 128 + 64, csl],
                            roE[hl * 64:(hl + 1) * 64, :])
                        nc.sync.dma_start(
                            dst[hl * 128 + 64:(hl + 1) * 128, csl],
                            roO[hl * 64:(hl + 1) * 64, :])

                def v_mms(xcA, xcB):
                    out = []
                    for mt in range(2):
                        m = 4 + mt
                        pvA = qkps.tile([128, CH], F32, tag="qk", name="pvA")
                        pvB = qkps.tile([128, CH], F32, tag="qk", name="pvB")
                        for d in range(NDT):
                            wsl = wq_sb[:, d * w3 + m * 128:d * w3 + (m + 1) * 128]
                            nc.tensor.matmul(pvA[:], wsl, xcA[:, d * CH:(d + 1) * CH],
                                             start=(d == 0), stop=(d == NDT - 1))
                            nc.tensor.matmul(pvB[:], wsl, xcB[:, d * CH:(d + 1) * CH],
                                             start=(d == 0), stop=(d == NDT - 1))
                        out.append((pvA, pvB))
                    return out

                def v_store(pvA, pvB, mt, cA, cB):
                    vtp = vps.tile([128, 1024], BF, tag="vtp")
                    for vi, pv in enumerate((pvA, pvB)):
                        vtmp = p1ss.tile([128, CH], BF, tag="vtmp")
                        nc.scalar.copy(vtmp[:], pv[:])
                        for ts in range(4):
                            nc.tensor.transpose(
                                vtp[:, vi * 512 + ts * 128:
                                    vi * 512 + (ts + 1) * 128],
                                vtmp[:, ts * 128:(ts + 1) * 128], ident[:])
                    vsb = p1ss.tile([128, 1024], BF, tag="vsb")
                    nc.vector.tensor_copy(vsb[:], vtp[:])
                    for vi, c in enumerate((cA, cB)):
                        nc.sync.dma_start(
                            v_dram[:, mt, c * 4:(c + 1) * 4, :],
                            vsb[:, vi * 512:(vi + 1) * 512].rearrange(
                                "p (a d) -> p a d", a=4))

                for p in range(NCH // 2):
                    cA, cB = 2 * p, 2 * p + 1
                    xcA, xcB = xc_pref
                    if cB + 1 < NCH:
                        xc_pref = [load_xc(cB + 1),
                                   load_xc(cB + 2) if cB + 2 < NCH else None]
                    # q matmuls, then k matmuls (PE stays busy while DVE
                    # applies rope to the q tiles), then v.
                    qp = qk_mms(xcA, xcB, 0)
                    for ci, c in enumerate((cA, cB)):
                        rope(qp[ci], 0, c)
                    kp = qk_mms(xcA, xcB, 1)
                    for ci, c in enumerate((cA, cB)):
                        sumsq(qp[ci], 0, c)
                        rope(kp[ci], 1, c)
                    vp = v_mms(xcA, xcB)
                    for ci, c in enumerate((cA, cB)):
                        sumsq(kp[ci], 1, c)
                    for mt in range(2):
                        pvA, pvB = vp[mt]
                        v_store(pvA, pvB, mt, cA, cB)

                    if cB % 4 == 3:
                        # batch bb fully projected: AllReduce its sumsq.
                        # (the 1/rms math runs in the phase-2 pools so the
                        # phase-1 pool close doesn't wait on the collective)
                        bb = cB // 4
                        nc.gpsimd.collective_compute(
                            "AllReduce", mybir.AluOpType.add,
                            replica_groups=[list(range(NC))],
                            ins=[ss_in[bb].opt()], outs=[ss_out[bb].opt()])

            # ---------------- Phase 2 + 3: attention, AG, wo ----------------

            with tc.tile_pool(name="wres2", bufs=1) as wres2, \
                 tc.tile_pool(name="sp", bufs=2, space="PSUM") as sp, \
                 tc.tile_pool(name="cxp", bufs=1, space="PSUM") as cxp, \
                 tc.tile_pool(name="mis", bufs=2, space="PSUM") as mis, \
                 tc.tile_pool(name="asb", bufs=2) as asb, \
                 tc.tile_pool(name="wsb", bufs=2) as wsb:

                # pending low-priority work (epilogues, wo slices): emitted
                # into later matmul streams so the PE never stalls on them.
                pend = []

                def flush_pend(n=99):
                    while pend and n > 0:
                        pend.pop(0)()
                        n -= 1

                ssf_done = set()

                def ssf_calc(b):
                    """1/rms rows for batch b ([128, 32] fold: partitions
                    0-63 q tokens, 64-127 k tokens)."""
                    FW = 2 * L // 128
                    ssf = asb.tile([128, FW], F32, tag="ssf", bufs=2)
                    nc.sync.dma_start(
                        ssf[:],
                        ss_out[b].rearrange("a (p f) -> (a p) f", p=64)[:, :])
                    nc.scalar.activation(ssf[:], ssf[:],
                                         mybir.ActivationFunctionType.Identity,
                                         bias=eps2[:], scale=1.0 / D)
                    nc.vector.reciprocal(ssf[:], ssf[:])
                    nc.scalar.activation(ssf[:], ssf[:],
                                         mybir.ActivationFunctionType.Sqrt,
                                         bias=0.0, scale=sc2[:])
                    nc.sync.dma_start(
                        ss_fin[b].rearrange("a (p f) -> (a p) f", p=64)[:, :],
                        ssf[:])

                def head_state(b, hl):
                    """Load and scale q, load k/v for head hl of batch b."""
                    if b not in ssf_done:
                        ssf_done.add(b)
                        ssf_calc(b)
                    bsl = slice(b * L, (b + 1) * L)
                    sqs = asb.tile([1, L], F32, tag="sqs", bufs=4)
                    nc.sync.dma_start(sqs[:], ss_fin[b][0:1, :])
                    skc = asb.tile([128, NJ], F32, tag="skc", bufs=4)
                    nc.sync.dma_start(
                        skc[:],
                        ss_fin[b][1:2, :].rearrange("a (j p) -> (a p) j", p=128))
                    qn = asb.tile([128, L], BF, tag="qn", bufs=4)
                    nc.sync.dma_start(qn[:], q_dram[hl * 128:(hl + 1) * 128, bsl])
                    kn = asb.tile([128, L], BF, tag="kn", bufs=4)
                    nc.sync.dma_start(kn[:], k_dram[hl * 128:(hl + 1) * 128, bsl])
                    vh = asb.tile([128, NJ * 128], BF, tag="vh", bufs=4)
                    nc.sync.dma_start(
                        vh.rearrange("p (a q) -> p a q", a=NJ),
                        v_dram[:, hl, b * NJ:(b + 1) * NJ, :])
                    # q-side 1/rms broadcast via fp32r K=1 matmuls, applied
                    # in place from PSUM.
                    for ii in range(L // 512):
                        bp = mis.tile([128, 512], F32, tag="mis", name="bp")
                        nc.tensor.matmul(
                            bp[:],
                            ones_r[:].bitcast(F32R),
                            sqs[0:1, ii * 512:(ii + 1) * 512].bitcast(F32R),
                            start=True, stop=True)
                        nc.vector.tensor_mul(
                            qn[:, ii * 512:(ii + 1) * 512],
                            qn[:, ii * 512:(ii + 1) * 512], bp[:])
                    return dict(qn=qn, kn=kn, vh=vh, skc=skc)

                def attn_g(b, hl, g, st):
                    """One 1024-col i-group of SDPA for head hl of batch b."""
                    qn, kn, vh, skc = st["qn"], st["kn"], st["vh"], st["skc"]
                    isl = slice(g * 1024, (g + 1) * 1024)
                    cps = None
                    dacc = [None, None]
                    pts = {}
                    for j in range(NJ):
                        ksl = kn[:, j * 128:(j + 1) * 128]
                        sps = sp.tile([128, 1024], F32, tag="sps", name="sps")
                        nc.tensor.matmul(
                            sps[:, 0:512], ksl,
                            qn[:, g * 1024:g * 1024 + 512],
                            start=True, stop=True)
                        nc.tensor.matmul(
                            sps[:, 512:1024], ksl,
                            qn[:, g * 1024 + 512:(g + 1) * 1024],
                            start=True, stop=True)
                        pt = asb.tile([128, 1024], BF, tag="pt", name="pt", bufs=4)
                        nc.scalar.activation(pt[:], sps[:],
                                             mybir.ActivationFunctionType.Exp,
                                             scale=skc[:, j:j + 1])
                        pts[j] = pt
                        e = j % 2
                        if j < 2:
                            dacc[e] = asb.tile([128, 1024], BF, tag=f"da{e}",
                                               name=f"da{e}", bufs=2)
                            nc.vector.tensor_copy(dacc[e][:], pt[:])
                        else:
                            eng = nc.vector if e == 0 else nc.gpsimd
                            eng.tensor_add(dacc[e][:], dacc[e][:], pt[:])
                        flush_pend(1)
                        if j > 0:
                            ptp = pts.pop(j - 1)
                            if j == 1:
                                cps = cxp.tile([128, 1024], F32, tag="cx")
                            vsl = vh[:, (j - 1) * 128:j * 128]
                            nc.tensor.matmul(cps[:, 0:512], vsl, ptp[:, 0:512],
                                             start=(j == 1), stop=False)
                            nc.tensor.matmul(cps[:, 512:1024], vsl,
                                             ptp[:, 512:1024],
                                             start=(j == 1), stop=False)
                    ptp = pts.pop(NJ - 1)
                    vsl = vh[:, (NJ - 1) * 128:NJ * 128]
                    nc.tensor.matmul(cps[:, 0:512], vsl, ptp[:, 0:512],
                                     start=False, stop=True)
                    nc.tensor.matmul(cps[:, 512:1024], vsl, ptp[:, 512:1024],
                                     start=False, stop=True)
                    # evacuate ctx to SBUF promptly so the single cx PSUM
                    # slot is free for the next group
                    ctxs = asb.tile([128, 1024], F32, tag="ctxs", bufs=2)
                    nc.vector.tensor_copy(ctxs[:], cps[:])

                    def epilogue():
                        df = asb.tile([128, 1024], BF, tag="df")
                        nc.vector.tensor_add(df[:], dacc[0][:], dacc[1][:])
                        for ic in range(2):
                            icl = slice(ic * 512, (ic + 1) * 512)
                            dn = mis.tile([128, 512], F32, tag="mis", name="dn")
                            nc.tensor.matmul(dn[0:1, :], ones_cb[:],
                                             df[:, icl], start=True, stop=True)
                            rr = asb.tile([1, 512], BF, tag="rr", bufs=2)
                            with nc.allow_low_precision("softmax denom bf16"):
                                nc.vector.reciprocal(rr[:], dn[0:1, :])
                            # broadcast 1/den over partitions, overwriting dn
                            nc.tensor.matmul(dn[:], ones_rb[:], rr[:],
                                             start=True, stop=True)
                            csb = asb.tile([128, 512], BF, tag="csb", bufs=3)
                            nc.vector.tensor_mul(csb[:], ctxs[:, icl], dn[:])
                            nc.sync.dma_start(
                                ctx_b[b][g][hl * 128:(hl + 1) * 128, icl],
                                csb[:])
                    pend.append(epilogue)

                def ag_half(b, h):
                    nc.gpsimd.collective_compute(
                        "AllGather", mybir.AluOpType.bypass,
                        replica_groups=[list(range(NC))],
                        ins=[ctx_b[b][h].opt()], outs=[ctx_g[b][h].opt()])

                def wo_pair(b, ip):
                    """wo for token cols ip*1024..(ip+1)*1024 of batch b,
                    emitted as 8 small d-slices via the pending queue so the
                    PE stream interleaves them with attention matmuls."""
                    cgr = ctx_g[b][ip].rearrange("(a p) t -> p a t", p=128)
                    cg = wsb.tile([128, NDT * 1024], BF, tag="cg")
                    nc.sync.dma_start(
                        cg.rearrange("p (a t) -> p a t", a=NDT), cgr[:, :, :])

                    for m in range(2):
                        cell = {}

                        def d_slice(m, dlo, dhi, cell=None):
                            def emit():
                                if dlo == 0:
                                    cell["opA"] = mis.tile(
                                        [128, 512], F32, tag="mis", name="wopA")
                                    cell["opB"] = mis.tile(
                                        [128, 512], F32, tag="mis", name="wopB")
                                opA, opB = cell["opA"], cell["opB"]
                                for d in range(dlo, dhi):
                                    wsl = wo_sb[:, d * DQ + m * 128:
                                                d * DQ + (m + 1) * 128]
                                    nc.tensor.matmul(
                                        opA[:], wsl,
                                        cg[:, d * 1024:d * 1024 + 512],
                                        start=(d == 0), stop=(d == NDT - 1))
                                    nc.tensor.matmul(
                                        opB[:], wsl,
                                        cg[:, d * 1024 + 512:(d + 1) * 1024],
                                        start=(d == 0), stop=(d == NDT - 1))
                                if dhi == NDT:
                                    osb = wsb.tile([128, 1024], F32, tag="osb")
                                    nc.vector.tensor_copy(osb[:, 0:512], opA[:])
                                    nc.vector.tensor_copy(osb[:, 512:1024],
                                                          opB[:])
                                    c0 = b * L + ip * 1024
                                    nc.sync.dma_start(
                                        outT[m * 128:(m + 1) * 128,
                                             c0:c0 + 1024],
                                        osb[:])
                            return emit
                        for dlo in range(0, NDT, 2):
                            pend.append(d_slice(m, dlo, dlo + 2, cell))

                sts = {}
                sts[(0, 0)] = head_state(0, 0)
                sts[(0, 1)] = head_state(0, 1)
                # wo weights are first needed ~90us in; load after the
                # first heads' q/k/v DMAs are queued
                wo_sb = wres2.tile([128, NDT * DQ], BF, tag="wo_sb")
                nc.sync.dma_start(
                    wo_sb.rearrange("p (a w) -> p a w", a=NDT),
                    woT.rearrange("(a p) w -> p a w", p=128))

                def sched_head(b, hl):
                    def emit():
                        sts[(b, hl)] = head_state(b, hl)
                    return emit

                for b in range(B - 1):
                    st0, st1 = sts[(b, 0)], sts[(b, 1)]
                    attn_g(b, 0, 0, st0)
                    attn_g(b, 1, 0, st1)
                    flush_pend()            # epilogues for half 0
                    ag_half(b, 0)
                    pend.append(sched_head(b + 1, 0))
                    pend.append(sched_head(b + 1, 1))
                    if b >= 1:
                        wo_pair(b - 1, 1)   # AG(b-1,1) is ~a phase old now
                    attn_g(b, 0, 1, st0)
                    attn_g(b, 1, 1, st1)
                    flush_pend()            # epilogues for half 1
                    ag_half(b, 1)
                    wo_pair(b, 0)           # flushed during next batch's g0
                # last batch: halves swapped (the g1 AG fires early), then
                # three deferred wo pairs fill the final AGs' flight time
                st0, st1 = sts[(B - 1, 0)], sts[(B - 1, 1)]
                attn_g(B - 1, 0, 1, st0)
                attn_g(B - 1, 1, 1, st1)
                flush_pend()
                ag_half(B - 1, 1)
                attn_g(B - 1, 0, 0, st0)
                attn_g(B - 1, 1, 0, st1)
                flush_pend()
                ag_half(B - 1, 0)
                wo_pair(B - 2, 1)           # data ancient: covers AG flight
                wo_pair(B - 1, 1)           # its AG fired a g-phase ago
                wo_pair(B - 1, 0)
                flush_pend()

    nc.compile()
    return nc


def _prep_inputs(x_BLD, freqs, wqkv, wo, q_norm_w, k_norm_w):
    """Host-side sharding/layout. Returns in_maps (list of 8 dicts)."""
    x = np.asarray(x_BLD, np.float32)
    freqs = np.asarray(freqs, np.float32)
    wqkv = np.asarray(wqkv, np.float32)
    wo = np.asarray(wo, np.float32)
    qw = np.asarray(q_norm_w, np.float32)
    kw = np.asarray(k_norm_w, np.float32)

    xT = np.ascontiguousarray(x.reshape(T, D).T).astype(BF_NP)
    sinT = np.ascontiguousarray(freqs[0].T)  # [D/2, L]
    cosT = np.ascontiguousarray(freqs[1].T)

    evens = 2 * np.arange(64)
    odds = evens + 1

    in_maps = []
    for r in range(NC):
        heads = [HPC * r + hl for hl in range(HPC)]
        # q/k row order: [h0 evens, h1 evens, h0 odds, h1 odds]
        qrows = np.concatenate([h * HD + evens for h in heads]
                               + [h * HD + odds for h in heads])
        rows = np.concatenate([qrows, D + qrows, 2 * D + DQ * r + np.arange(DQ)])
        wqkvT = np.ascontiguousarray(wqkv[rows, :].T).astype(BF_NP)
        woT = np.ascontiguousarray(wo[DQ * r:DQ * (r + 1), :].T).astype(BF_NP)

        tabs = {p + sfx: np.empty((HD, L), np.float32)
                for p in ("tq", "tk") for sfx in ("ce", "so", "co", "se")}
        for hl, h in enumerate(heads):
            rsl = slice(hl * 64, (hl + 1) * 64)
            cosP = cosT[h * 64:(h + 1) * 64]
            sinP = sinT[h * 64:(h + 1) * 64]
            for w, p in ((qw, "tq"), (kw, "tk")):
                w_e = w[h * HD + evens][:, None]
                w_o = w[h * HD + odds][:, None]
                tabs[p + "ce"][rsl] = w_e * cosP
                tabs[p + "so"][rsl] = w_o * sinP
                tabs[p + "co"][rsl] = w_o * cosP
                tabs[p + "se"][rsl] = w_e * sinP

        in_maps.append({
            "xT": xT,
            "wqkvT": wqkvT,
            "woT": woT,
            **{k: v.astype(BF_NP) for k, v in tabs.items()},
        })
    return in_maps


def _assemble(results):
    outT = np.empty((D, T), np.float32)
    for r in range(NC):
        outT[DQ * r:DQ * (r + 1)] = results[r]["outT"]
    return np.ascontiguousarray(outT.T).reshape(B, L, D)


def _install_ntff_hook():
    """The agent image's antenv lacks axon_hooks; provide the documented shim
    so run_bass_kernel_spmd(trace=True) can NTFF-profile via libaxon_pjrt."""
    try:
        import antenv.axon_hooks  # noqa: F401
        return
    except ImportError:
        pass
    import types
    hookf = None
    try:
        from trn_agent_boot.trn_boot import _ntff_profile_via_ctypes
        hookf = _ntff_profile_via_ctypes("/opt/axon/libaxon_pjrt.so")
    except Exception:
        pass
    mod = types.ModuleType("antenv.axon_hooks")
    state = {"h": hookf}
    mod.set_axon_ntff_profile_hook = lambda h: state.__setitem__("h", h)
    mod.get_axon_ntff_profile_hook = lambda: state["h"]
    sys.modules["antenv.axon_hooks"] = mod
    import antenv
    antenv.axon_hooks = mod


def kernel(x_BLD, freqs, wqkv, wo, q_norm_w, k_norm_w, _trace=False):
    from concourse.bass_utils import run_bass_kernel_spmd
    if _trace:
        _install_ntff_hook()
    if "nc" not in _CACHE:
        _CACHE["nc"] = build_nc()
    nc = _CACHE["nc"]
    in_maps = _prep_inputs(x_BLD, freqs, wqkv, wo, q_norm_w, k_norm_w)
    res = run_bass_kernel_spmd(nc, in_maps, core_ids=list(range(NC)),
                               trace=_trace)
    out = _assemble(res.results)
    if _trace:
        return out, res
    return out


# revision 31
# speedup vs baseline: 1.0518x; 1.0518x over previous
"""Distributed Trainium2 kernel for nn_Attention_79207786873592.

Full attention block: qkv projection -> RMSNorm(q,k) -> RoPE -> SDPA -> wo.
B=4, L=2048, D=2048, H=16 heads, head_dim=128, fp32 I/O (bf16 compute).

Sharding: 8-way tensor-parallel over heads. Each core owns 2 heads:
  - computes its 768-row slice of the qkv projection,
  - a tiny AllReduce (64KB) produces the global per-token sum-of-squares for
    the full-D RMSNorm of q and k,
  - local RoPE + SDPA on its 2 heads,
  - AllGather of the per-head attention context (per batch half, pipelined),
  - local 256-column slice of the wo projection.
Host side pre-transposes x and the weights into [contract_dim, out] layouts so
no on-device transposes are needed, and folds the norm weights into per-head
RoPE tables. Within each head, q/k output dims are permuted (evens, then odds)
so RoPE pair mixing becomes partition-block operations on the vector engine.

Perf structure:
  - qkv processes chunks in pairs sharing each stationary weight load.
  - scores for two 512-col i-chunks land in one [128,1024] PSUM pair tile so
    a single EXP instruction covers both; softmax partials accumulate on the
    vector engine (even tiles) and gpsimd (odd tiles).
  - softmax epilogues (denominator reduce / reciprocal / normalize) are
    deferred and emitted inside the next block's matmul stream so the
    in-order PE queue never stalls on vector/scalar results.
  - broadcasts use fp32r / bf16 matmuls (1 cycle/row instead of 4).
  - wo is emitted in slices interleaved with the next batch's attention and
    consumes per-half AllGathers to shorten the tail.
"""

import sys

sys.path.insert(0, "/opt/trn_rl_repo")

import numpy as np
import ml_dtypes

import concourse.bass as bass
import concourse.tile as tile
import concourse.mybir as mybir
from concourse import bacc

B, L, D, H = 4, 2048, 2048, 16
HD = D // H              # 128
NC = 8                   # cores
HPC = H // NC            # 2 heads per core
DQ = HPC * HD            # 256 rows of q/k/v per core
T = B * L                # 8192 tokens
EPS = 1e-5
CH = 512                 # phase-1 token chunk
NCH = T // CH            # 16
NDT = D // 128           # 16 contraction tiles
NJ = L // 128            # 16 k-tiles per batch
BF = mybir.dt.bfloat16
F32 = mybir.dt.float32
F32R = mybir.dt.float32r
BF_NP = ml_dtypes.bfloat16

_CACHE = {}


def build_nc():
    nc = bacc.Bacc("TRN2", target_bir_lowering=False, debug=False, num_devices=NC)

    xT = nc.dram_tensor("xT", [D, T], BF, kind="ExternalInput").ap()
    wqkvT = nc.dram_tensor("wqkvT", [D, 3 * DQ], BF, kind="ExternalInput").ap()
    woT = nc.dram_tensor("woT", [D, DQ], BF, kind="ExternalInput").ap()
    TAB_NAMES = [p + sfx for p in ("tq", "tk")
                 for sfx in ("ce", "so", "co", "se")]
    tab_ext = {nm: nc.dram_tensor(nm, [HD, L], BF, kind="ExternalInput").ap()
               for nm in TAB_NAMES}
    outT = nc.dram_tensor("outT", [DQ, T], F32, kind="ExternalOutput").ap()

    with tile.TileContext(nc) as tc:
        with tc.tile_pool(name="dram", bufs=1, space="DRAM") as dram, \
             tc.tile_pool(name="consts", bufs=1) as consts:

            q_dram = dram.tile([DQ, T], BF, tag="q_dram")
            k_dram = dram.tile([DQ, T], BF, tag="k_dram")
            # v in [partition, head, token-tile, head-dim] layout so both the
            # phase-1 writes and the per-head attention reads are contiguous
            v_dram = dram.tile([128, HPC, T // 128, 128], BF, tag="v_dram")
            ss_in = [dram.tile([2, L], F32, tag=f"ss_in{b}", name=f"ss_in{b}")
                     for b in range(B)]
            ss_out = [dram.tile([2, L], F32, tag=f"ss_out{b}", name=f"ss_out{b}",
                                addr_space="Shared") for b in range(B)]
            ss_fin = [dram.tile([2, L], F32, tag=f"ss_fin{b}", name=f"ss_fin{b}")
                      for b in range(B)]
            # per-half context buffers: half h covers token cols h*1024..
            ctx_b = [[dram.tile([DQ, L // 2], BF, tag=f"ctxb{b}_{h}",
                                name=f"ctxb{b}_{h}") for h in range(2)]
                     for b in range(B)]
            ctx_g = [[dram.tile([NC * DQ, L // 2], BF, tag=f"ctxg{b}_{h}",
                                name=f"ctxg{b}_{h}", addr_space="Shared")
                      for h in range(2)] for b in range(B)]

            ones_c = consts.tile([128, 1], F32, tag="ones_c")
            nc.vector.memset(ones_c[:], 1.0)
            ones_cb = consts.tile([128, 1], BF, tag="ones_cb")
            nc.vector.memset(ones_cb[:], 1.0)
            ones_r = consts.tile([1, 128], F32, tag="ones_r")
            nc.vector.memset(ones_r[:], 1.0)
            ones_rb = consts.tile([1, 128], BF, tag="ones_rb")
            nc.vector.memset(ones_rb[:], 1.0)
            sc2 = consts.tile([128, 1], F32, tag="sc2")
            nc.vector.memset(sc2[0:64, :], 1.0 / HD)
            nc.vector.memset(sc2[64:128, :], 1.0)
            eps2 = consts.tile([128, 1], F32, tag="eps2")
            nc.vector.memset(eps2[:], EPS)
            ident = consts.tile([128, 128], BF, tag="ident")
            from concourse.masks import make_identity
            make_identity(nc, ident[:])

            # ---------------- Phase 1: qkv + sumsq + rope ----------------
            with tc.tile_pool(name="wres1", bufs=1) as wres1, \
                 tc.tile_pool(name="xc", bufs=4) as xcp, \
                 tc.tile_pool(name="p1sb", bufs=2) as p1sb, \
                 tc.tile_pool(name="p1ss", bufs=3) as p1ss, \
                 tc.tile_pool(name="qkps", bufs=6, space="PSUM") as qkps, \
                 tc.tile_pool(name="vps", bufs=1, space="PSUM") as vps, \
                 tc.tile_pool(name="ssps", bufs=1, space="PSUM") as ssps:

                # resident qkv weights; issue after nothing, before tables,
                # and prefetch the first two x chunks before the tables so
                # the first matmul starts as early as possible.
                wq_sb = wres1.tile([128, NDT * 3 * DQ], BF, tag="wq_sb")
                nc.sync.dma_start(
                    wq_sb.rearrange("p (a w) -> p a w", a=NDT),
                    wqkvT.rearrange("(a p) w -> p a w", p=128))

                xTr = xT.rearrange("(a p) t -> p a t", p=128)

                def load_xc(c):
                    xc = xcp.tile([128, NDT * CH], BF, tag="xc", name=f"xc{c % 4}")
                    nc.sync.dma_start(
                        xc.rearrange("p (a t) -> p a t", a=NDT),
                        xTr[:, :, c * CH:(c + 1) * CH])
                    return xc

                xc_pref = [load_xc(0), load_xc(1)]

                tabs = {}
                for nm, t in tab_ext.items():
                    tt = wres1.tile([128, L], BF, tag=nm + "_sb", name=nm + "_sb")
                    nc.sync.dma_start(tt[:], t[:, :])
                    tabs[nm] = tt

                w3 = 3 * DQ

                def qk_mms(xcA, xcB, side):
                    """Emit q-or-k matmuls for a chunk pair; returns
                    [psE_A, psE_B, psO_A, psO_B]."""
                    out = []
                    for eo in range(2):
                        m = side * 2 + eo
                        psA = qkps.tile([128, CH], F32, tag="qk", name="psA")
                        psB = qkps.tile([128, CH], F32, tag="qk", name="psB")
                        for d in range(NDT):
                            wsl = wq_sb[:, d * w3 + m * 128:d * w3 + (m + 1) * 128]
                            nc.tensor.matmul(psA[:], wsl, xcA[:, d * CH:(d + 1) * CH],
                                             start=(d == 0), stop=(d == NDT - 1))
                            nc.tensor.matmul(psB[:], wsl, xcB[:, d * CH:(d + 1) * CH],
                                             start=(d == 0), stop=(d == NDT - 1))
                        out.append((psA, psB))
                    (psEA, psEB), (psOA, psOB) = out
                    return [(psEA, psOA), (psEB, psOB)]

                def sumsq(ps_pair, side, c):
                    psE, psO = ps_pair
                    ssp = ssps.tile([1, CH], F32, tag="ssp", name="ssp")
                    for eo, ps in enumerate((psE, psO)):
                        sq = p1sb.tile([128, CH], BF, tag="sq", bufs=3)
                        nc.scalar.square(sq[:], ps[:])
                        nc.tensor.matmul(ssp[:], ones_cb[:], sq[:],
                                         start=(eo == 0), stop=(eo == 1))
                    ssr = p1sb.tile([1, CH], F32, tag="ssr", bufs=2)
                    nc.scalar.copy(ssr[:], ssp[:])
                    nc.sync.dma_start(
                        ss_in[c // 4][side:side + 1,
                                      (c % 4) * CH:(c % 4 + 1) * CH],
                        ssr[:])

                def rope(ps_pair, side, c):
                    psE, psO = ps_pair
                    lc = c % 4
                    tsl = slice(lc * CH, (lc + 1) * CH)
                    csl = slice(c * CH, (c + 1) * CH)
                    pre = "tq" if side == 0 else "tk"
                    ta = p1sb.tile([128, CH], F32, tag="ta")
                    nc.vector.tensor_mul(ta[:], psE[:], tabs[pre + "ce"][:, tsl])
                    tb = p1sb.tile([128, CH], F32, tag="tb")
                    nc.vector.tensor_mul(tb[:], psO[:], tabs[pre + "so"][:, tsl])
                    roE = p1sb.tile([128, CH], BF, tag="roE")
                    nc.vector.tensor_sub(roE[:], ta[:], tb[:])
                    tc_ = p1sb.tile([128, CH], F32, tag="tc_")
                    nc.vector.tensor_mul(tc_[:], psO[:], tabs[pre + "co"][:, tsl])
                    td = p1sb.tile([128, CH], F32, tag="td")
                    nc.vector.tensor_mul(td[:], psE[:], tabs[pre + "se"][:, tsl])
                    roO = p1sb.tile([128, CH], BF, tag="roO")
                    nc.vector.tensor_add(roO[:], tc_[:], td[:])
                    dst = q_dram if side == 0 else k_dram
                    for hl in range(HPC):
                        nc.sync.dma_start(
                            dst[hl * 128:hl * 128 + 64, csl],
                            roE[hl * 64:(hl + 1) * 64, :])
                        nc.sync.dma_start(
                            dst[hl * 128 + 64:(hl + 1) * 128, csl],
                            roO[hl * 64:(hl + 1) * 64, :])

                def v_mms(xcA, xcB):
                    out = []
                    for mt in range(2):
                        m = 4 + mt
                        pvA = qkps.tile([128, CH], F32, tag="qk", name="pvA")
                        pvB = qkps.tile([128, CH], F32, tag="qk", name="pvB")
                        for d in range(NDT):
                            wsl = wq_sb[:, d * w3 + m * 128:d * w3 + (m + 1) * 128]
                            nc.tensor.matmul(pvA[:], wsl, xcA[:, d * CH:(d + 1) * CH],
                                             start=(d == 0), stop=(d == NDT - 1))
                            nc.tensor.matmul(pvB[:], wsl, xcB[:, d * CH:(d + 1) * CH],
                                             start=(d == 0), stop=(d == NDT - 1))
                        out.append((pvA, pvB))
                    return out

                def v_store(pvA, pvB, mt, cA, cB):
                    vtp = vps.tile([128, 1024], BF, tag="vtp")
                    for vi, pv in enumerate((pvA, pvB)):
                        vtmp = p1ss.tile([128, CH], BF, tag="vtmp")
                        nc.scalar.copy(vtmp[:], pv[:])
                        for ts in range(4):
                            nc.tensor.transpose(
                                vtp[:, vi * 512 + ts * 128:
                                    vi * 512 + (ts + 1) * 128],
                                vtmp[:, ts * 128:(ts + 1) * 128], ident[:])
                    vsb = p1ss.tile([128, 1024], BF, tag="vsb")
                    nc.vector.tensor_copy(vsb[:], vtp[:])
                    for vi, c in enumerate((cA, cB)):
                        nc.sync.dma_start(
                            v_dram[:, mt, c * 4:(c + 1) * 4, :],
                            vsb[:, vi * 512:(vi + 1) * 512].rearrange(
                                "p (a d) -> p a d", a=4))

                for p in range(NCH // 2):
                    cA, cB = 2 * p, 2 * p + 1
                    xcA, xcB = xc_pref
                    if cB + 1 < NCH:
                        xc_pref = [load_xc(cB + 1),
                                   load_xc(cB + 2) if cB + 2 < NCH else None]
                    # q matmuls, then k matmuls (PE stays busy while DVE
                    # applies rope to the q tiles), then v.
                    qp = qk_mms(xcA, xcB, 0)
                    for ci, c in enumerate((cA, cB)):
                        rope(qp[ci], 0, c)
                    kp = qk_mms(xcA, xcB, 1)
                    for ci, c in enumerate((cA, cB)):
                        sumsq(qp[ci], 0, c)
                        rope(kp[ci], 1, c)
                    vp = v_mms(xcA, xcB)
                    for ci, c in enumerate((cA, cB)):
                        sumsq(kp[ci], 1, c)
                    for mt in range(2):
                        pvA, pvB = vp[mt]
                        v_store(pvA, pvB, mt, cA, cB)

                    if cB % 4 == 3:
                        # batch bb fully projected: AllReduce its sumsq.
                        # (the 1/rms math runs in the phase-2 pools so the
                        # phase-1 pool close doesn't wait on the collective)
                        bb = cB // 4
                        nc.gpsimd.collective_compute(
                            "AllReduce", mybir.AluOpType.add,
                            replica_groups=[list(range(NC))],
                            ins=[ss_in[bb].opt()], outs=[ss_out[bb].opt()])

            # ---------------- Phase 2 + 3: attention, AG, wo ----------------

            with tc.tile_pool(name="wres2", bufs=1) as wres2, \
                 tc.tile_pool(name="sp", bufs=2, space="PSUM") as sp, \
                 tc.tile_pool(name="cxp", bufs=1, space="PSUM") as cxp, \
                 tc.tile_pool(name="mis", bufs=2, space="PSUM") as mis, \
                 tc.tile_pool(name="asb", bufs=2) as asb, \
                 tc.tile_pool(name="wsb", bufs=2) as wsb:

                # pending low-priority work (epilogues, wo slices): emitted
                # into later matmul streams so the PE never stalls on them.
                pend = []

                def flush_pend(n=99):
                    while pend and n > 0:
                        pend.pop(0)()
                        n -= 1

                ssf_done = set()

                def ssf_calc(b):
                    """1/rms rows for batch b ([128, 32] fold: partitions
                    0-63 q tokens, 64-127 k tokens)."""
                    FW = 2 * L // 128
                    ssf = asb.tile([128, FW], F32, tag="ssf", bufs=2)
                    nc.sync.dma_start(
                        ssf[:],
                        ss_out[b].rearrange("a (p f) -> (a p) f", p=64)[:, :])
                    nc.scalar.activation(ssf[:], ssf[:],
                                         mybir.ActivationFunctionType.Identity,
                                         bias=eps2[:], scale=1.0 / D)
                    nc.vector.reciprocal(ssf[:], ssf[:])
                    nc.scalar.activation(ssf[:], ssf[:],
                                         mybir.ActivationFunctionType.Sqrt,
                                         bias=0.0, scale=sc2[:])
                    nc.sync.dma_start(
                        ss_fin[b].rearrange("a (p f) -> (a p) f", p=64)[:, :],
                        ssf[:])

                def head_state(b, hl):
                    """Load and scale q, load k/v for head hl of batch b."""
                    if b not in ssf_done:
                        ssf_done.add(b)
                        ssf_calc(b)
                    bsl = slice(b * L, (b + 1) * L)
                    sqs = asb.tile([1, L], F32, tag="sqs", bufs=4)
                    nc.sync.dma_start(sqs[:], ss_fin[b][0:1, :])
                    skc = asb.tile([128, NJ], F32, tag="skc", bufs=4)
                    nc.sync.dma_start(
                        skc[:],
                        ss_fin[b][1:2, :].rearrange("a (j p) -> (a p) j", p=128))
                    qn = asb.tile([128, L], BF, tag="qn", bufs=4)
                    nc.sync.dma_start(qn[:], q_dram[hl * 128:(hl + 1) * 128, bsl])
                    kn = asb.tile([128, L], BF, tag="kn", bufs=4)
                    nc.sync.dma_start(kn[:], k_dram[hl * 128:(hl + 1) * 128, bsl])
                    vh = asb.tile([128, NJ * 128], BF, tag="vh", bufs=4)
                    nc.sync.dma_start(
                        vh.rearrange("p (a q) -> p a q", a=NJ),
                        v_dram[:, hl, b * NJ:(b + 1) * NJ, :])
                    # q-side 1/rms broadcast via fp32r K=1 matmuls, applied
                    # in place from PSUM.
                    for ii in range(L // 512):
                        bp = mis.tile([128, 512], F32, tag="mis", name="bp")
                        nc.tensor.matmul(
                            bp[:],
                            ones_r[:].bitcast(F32R),
                            sqs[0:1, ii * 512:(ii + 1) * 512].bitcast(F32R),
                            start=True, stop=True)
                        nc.vector.tensor_mul(
                            qn[:, ii * 512:(ii + 1) * 512],
                            qn[:, ii * 512:(ii + 1) * 512], bp[:])
                    return dict(qn=qn, kn=kn, vh=vh, skc=skc)

                def attn_g(b, hl, g, st):
                    """One 1024-col i-group of SDPA for head hl of batch b."""
                    qn, kn, vh, skc = st["qn"], st["kn"], st["vh"], st["skc"]
                    isl = slice(g * 1024, (g + 1) * 1024)
                    cps = None
                    dacc = [None, None]
                    pts = {}
                    for j in range(NJ):
                        ksl = kn[:, j * 128:(j + 1) * 128]
                        sps = sp.tile([128, 1024], F32, tag="sps", name="sps")
                        nc.tensor.matmul(
                            sps[:, 0:512], ksl,
                            qn[:, g * 1024:g * 1024 + 512],
                            start=True, stop=True)
                        nc.tensor.matmul(
                            sps[:, 512:1024], ksl,
                            qn[:, g * 1024 + 512:(g + 1) * 1024],
                            start=True, stop=True)
                        pt = asb.tile([128, 1024], BF, tag="pt", name="pt", bufs=4)
                        nc.scalar.activation(pt[:], sps[:],
                                             mybir.ActivationFunctionType.Exp,
                                             scale=skc[:, j:j + 1])
                        pts[j] = pt
                        e = j % 2
                        if j < 2:
                            dacc[e] = asb.tile([128, 1024], BF, tag=f"da{e}",
                                               name=f"da{e}", bufs=2)
                            nc.vector.tensor_copy(dacc[e][:], pt[:])
                        else:
                            # all on DVE: gpsimd tensor ops contend for SBUF
                            # ports and slow DVE/ACT by ~20%
                            nc.vector.tensor_add(dacc[e][:], dacc[e][:], pt[:])
                        flush_pend(1)
                        if j > 0:
                            ptp = pts.pop(j - 1)
                            if j == 1:
                                cps = cxp.tile([128, 1024], F32, tag="cx")
                            vsl = vh[:, (j - 1) * 128:j * 128]
                            nc.tensor.matmul(cps[:, 0:512], vsl, ptp[:, 0:512],
                                             start=(j == 1), stop=False)
                            nc.tensor.matmul(cps[:, 512:1024], vsl,
                                             ptp[:, 512:1024],
                                             start=(j == 1), stop=False)
                    ptp = pts.pop(NJ - 1)
                    vsl = vh[:, (NJ - 1) * 128:NJ * 128]
                    nc.tensor.matmul(cps[:, 0:512], vsl, ptp[:, 0:512],
                                     start=False, stop=True)
                    nc.tensor.matmul(cps[:, 512:1024], vsl, ptp[:, 512:1024],
                                     start=False, stop=True)
                    # evacuate ctx to SBUF promptly so the single cx PSUM
                    # slot is free for the next group
                    ctxs = asb.tile([128, 1024], F32, tag="ctxs", bufs=2)
                    nc.vector.tensor_copy(ctxs[:], cps[:])

                    def epilogue():
                        df = asb.tile([128, 1024], BF, tag="df")
                        nc.vector.tensor_add(df[:], dacc[0][:], dacc[1][:])
                        for ic in range(2):
                            icl = slice(ic * 512, (ic + 1) * 512)
                            dn = mis.tile([128, 512], F32, tag="mis", name="dn")
                            nc.tensor.matmul(dn[0:1, :], ones_cb[:],
                                             df[:, icl], start=True, stop=True)
                            rr = asb.tile([1, 512], BF, tag="rr", bufs=2)
                            with nc.allow_low_precision("softmax denom bf16"):
                                nc.vector.reciprocal(rr[:], dn[0:1, :])
                            # broadcast 1/den over partitions, overwriting dn
                            nc.tensor.matmul(dn[:], ones_rb[:], rr[:],
                                             start=True, stop=True)
                            csb = asb.tile([128, 512], BF, tag="csb", bufs=3)
                            nc.vector.tensor_mul(csb[:], ctxs[:, icl], dn[:])
                            nc.sync.dma_start(
                                ctx_b[b][g][hl * 128:(hl + 1) * 128, icl],
                                csb[:])
                    pend.append(epilogue)

                def ag_half(b, h):
                    nc.gpsimd.collective_compute(
                        "AllGather", mybir.AluOpType.bypass,
                        replica_groups=[list(range(NC))],
                        ins=[ctx_b[b][h].opt()], outs=[ctx_g[b][h].opt()])

                def wo_pair(b, ip):
                    """wo for token cols ip*1024..(ip+1)*1024 of batch b,
                    emitted as 8 small d-slices via the pending queue so the
                    PE stream interleaves them with attention matmuls."""
                    cgr = ctx_g[b][ip].rearrange("(a p) t -> p a t", p=128)
                    cg = wsb.tile([128, NDT * 1024], BF, tag="cg")
                    nc.sync.dma_start(
                        cg.rearrange("p (a t) -> p a t", a=NDT), cgr[:, :, :])

                    for m in range(2):
                        cell = {}

                        def d_slice(m, dlo, dhi, cell=None):
                            def emit():
                                if dlo == 0:
                                    cell["opA"] = mis.tile(
                                        [128, 512], F32, tag="mis", name="wopA")
                                    cell["opB"] = mis.tile(
                                        [128, 512], F32, tag="mis", name="wopB")
                                opA, opB = cell["opA"], cell["opB"]
                                for d in range(dlo, dhi):
                                    wsl = wo_sb[:, d * DQ + m * 128:
                                                d * DQ + (m + 1) * 128]
                                    nc.tensor.matmul(
                                        opA[:], wsl,
                                        cg[:, d * 1024:d * 1024 + 512],
                                        start=(d == 0), stop=(d == NDT - 1))
                                    nc.tensor.matmul(
                                        opB[:], wsl,
                                        cg[:, d * 1024 + 512:(d + 1) * 1024],
                                        start=(d == 0), stop=(d == NDT - 1))
                                if dhi == NDT:
                                    osb = wsb.tile([128, 1024], F32, tag="osb")
                                    nc.vector.tensor_copy(osb[:, 0:512], opA[:])
                                    nc.vector.tensor_copy(osb[:, 512:1024],
                                                          opB[:])
                                    c0 = b * L + ip * 1024
                                    nc.sync.dma_start(
                                        outT[m * 128:(m + 1) * 128,
                                             c0:c0 + 1024],
                                        osb[:])
                            return emit
                        for dlo in range(0, NDT, 2):
                            pend.append(d_slice(m, dlo, dlo + 2, cell))

                sts = {}
                sts[(0, 0)] = head_state(0, 0)
                sts[(0, 1)] = head_state(0, 1)
                # wo weights are first needed ~90us in; load after the
                # first heads' q/k/v DMAs are queued
                wo_sb = wres2.tile([128, NDT * DQ], BF, tag="wo_sb")
                nc.sync.dma_start(
                    wo_sb.rearrange("p (a w) -> p a w", a=NDT),
                    woT.rearrange("(a p) w -> p a w", p=128))

                def sched_head(b, hl):
                    def emit():
                        sts[(b, hl)] = head_state(b, hl)
                    return emit

                for b in range(B - 1):
                    st0, st1 = sts[(b, 0)], sts[(b, 1)]
                    attn_g(b, 0, 0, st0)
                    attn_g(b, 1, 0, st1)
                    flush_pend()            # epilogues for half 0
                    ag_half(b, 0)
                    pend.append(sched_head(b + 1, 0))
                    pend.append(sched_head(b + 1, 1))
                    if b >= 1:
                        wo_pair(b - 1, 1)   # AG(b-1,1) is ~a phase old now
                    attn_g(b, 0, 1, st0)
                    attn_g(b, 1, 1, st1)
                    flush_pend()            # epilogues for half 1
                    ag_half(b, 1)
                    wo_pair(b, 0)           # flushed during next batch's g0
                # last batch: halves swapped (the g1 AG fires early), then
                # three deferred wo pairs fill the final AGs' flight time
                st0, st1 = sts[(B - 1, 0)], sts[(B - 1, 1)]
                attn_g(B - 1, 0, 1, st0)
                attn_g(B - 1, 1, 1, st1)
                flush_pend()
                ag_half(B - 1, 1)
                attn_g(B - 1, 0, 0, st0)
                attn_g(B - 1, 1, 0, st1)
                flush_pend()
                ag_half(B - 1, 0)
                wo_pair(B - 2, 1)           # data ancient: covers AG flight
                wo_pair(B - 1, 1)           # its AG fired a g-phase ago
                wo_pair(B - 1, 0)
                flush_pend()

    nc.compile()
    return nc


def _prep_inputs(x_BLD, freqs, wqkv, wo, q_norm_w, k_norm_w):
    """Host-side sharding/layout. Returns in_maps (list of 8 dicts)."""
    x = np.asarray(x_BLD, np.float32)
    freqs = np.asarray(freqs, np.float32)
    wqkv = np.asarray(wqkv, np.float32)
    wo = np.asarray(wo, np.float32)
    qw = np.asarray(q_norm_w, np.float32)
    kw = np.asarray(k_norm_w, np.float32)

    xT = np.ascontiguousarray(x.reshape(T, D).T).astype(BF_NP)
    sinT = np.ascontiguousarray(freqs[0].T)  # [D/2, L]
    cosT = np.ascontiguousarray(freqs[1].T)

    evens = 2 * np.arange(64)
    odds = evens + 1

    in_maps = []
    for r in range(NC):
        heads = [HPC * r + hl for hl in range(HPC)]
        # q/k row order: [h0 evens, h1 evens, h0 odds, h1 odds]
        qrows = np.concatenate([h * HD + evens for h in heads]
                               + [h * HD + odds for h in heads])
        rows = np.concatenate([qrows, D + qrows, 2 * D + DQ * r + np.arange(DQ)])
        wqkvT = np.ascontiguousarray(wqkv[rows, :].T).astype(BF_NP)
        woT = np.ascontiguousarray(wo[DQ * r:DQ * (r + 1), :].T).astype(BF_NP)

        tabs = {p + sfx: np.empty((HD, L), np.float32)
                for p in ("tq", "tk") for sfx in ("ce", "so", "co", "se")}
        for hl, h in enumerate(heads):
            rsl = slice(hl * 64, (hl + 1) * 64)
            cosP = cosT[h * 64:(h + 1) * 64]
            sinP = sinT[h * 64:(h + 1) * 64]
            for w, p in ((qw, "tq"), (kw, "tk")):
                w_e = w[h * HD + evens][:, None]
                w_o = w[h * HD + odds][:, None]
                tabs[p + "ce"][rsl] = w_e * cosP
                tabs[p + "so"][rsl] = w_o * sinP
                tabs[p + "co"][rsl] = w_o * cosP
                tabs[p + "se"][rsl] = w_e * sinP

        in_maps.append({
            "xT": xT,
            "wqkvT": wqkvT,
            "woT": woT,
            **{k: v.astype(BF_NP) for k, v in tabs.items()},
        })
    return in_maps


def _assemble(results):
    outT = np.empty((D, T), np.float32)
    for r in range(NC):
        outT[DQ * r:DQ * (r + 1)] = results[r]["outT"]
    return np.ascontiguousarray(outT.T).reshape(B, L, D)


def _install_ntff_hook():
    """The agent image's antenv lacks axon_hooks; provide the documented shim
    so run_bass_kernel_spmd(trace=True) can NTFF-profile via libaxon_pjrt."""
    try:
        import antenv.axon_hooks  # noqa: F401
        return
    except ImportError:
        pass
    import types
    hookf = None
    try:
        from trn_agent_boot.trn_boot import _ntff_profile_via_ctypes
        hookf = _ntff_profile_via_ctypes("/opt/axon/libaxon_pjrt.so")
    except Exception:
        pass
    mod = types.ModuleType("antenv.axon_hooks")
    state = {"h": hookf}
    mod.set_axon_ntff_profile_hook = lambda h: state.__setitem__("h", h)
    mod.get_axon_ntff_profile_hook = lambda: state["h"]
    sys.modules["antenv.axon_hooks"] = mod
    import antenv
    antenv.axon_hooks = mod


def kernel(x_BLD, freqs, wqkv, wo, q_norm_w, k_norm_w, _trace=False):
    from concourse.bass_utils import run_bass_kernel_spmd
    if _trace:
        _install_ntff_hook()
    if "nc" not in _CACHE:
        _CACHE["nc"] = build_nc()
    nc = _CACHE["nc"]
    in_maps = _prep_inputs(x_BLD, freqs, wqkv, wo, q_norm_w, k_norm_w)
    res = run_bass_kernel_spmd(nc, in_maps, core_ids=list(range(NC)),
                               trace=_trace)
    out = _assemble(res.results)
    if _trace:
        return out, res
    return out
